# revision 1
# baseline (speedup 1.0000x reference)
"""Bass/Trainium2 kernel for nn_Attention_42305427865835.

Computes, for d_hidden [B,N,D], encoder_outputs [B,Lin,E], W1 [E+N*D, D],
b1 [D], w2 [D]:
    dec_proj = d_flat @ W1[:N*D] + b1                    # [B, D]
    enc_proj = enc @ W1[N*D:]                            # [B, Lin, E->D]
    scores   = tanh(enc_proj + dec_proj[:,None,:]) @ w2  # [B, Lin]
    out      = softmax(scores, axis=-1)
sharded data-parallel over batch, 4 batches per core on 8 cores.

Device-side layout is transposed ("T layout": D/E on partitions, Lin on the
free axis) so the contraction over E maps onto the PE array and the
dec_proj/b1 bias-add rides the ScalarE activation's per-partition bias.

The enc matmul (the dominant FLOPs) runs in fp8e4 with
MatmulPerfMode.DoubleRow: host pre-scales enc by 32 and W1_e by 8192
(keeping both inside fp8e4's +-240 range), packs the contraction as
[P, etile, free] so an e-tile PAIR is one K=256 DoubleRow matmul, and the
tanh activation's scale=2^-18 undoes the scaling exactly.  The score matmul
stays bf16 (fp8 there would blow the error budget).  Simulated end-to-end
absmax-relative error 1.83e-2 (gate 2e-2); the same simulator matches the
bf16 baseline's hardware error to 3 digits.

Softmax: scores for the 4 Lin-chunks of a batch land on PSUM partitions
{0,32,64,96} of one bank (tile_position picks the column group), so ONE Exp
activation covers the whole batch and its accum_out gives per-chunk sums.
The bank is memset to -100 first so unused partitions exp to 0, making the
ones-vector partition-sum matmul exact; gpsimd.partition_broadcast spreads
1/sum back across partitions for the final scale.  The partition-sum matmul
reuses element [0,0] of the score bank (no spare PSUM bank exists), and the
tail is pipelined across two chunk slots so the PE never waits on the Exp.

Score matmuls are emitted one chunk behind the enc matmuls so the PE queue
never head-blocks on the tanh that produces their input.  W1_d comes in two
half-tensors (d columns 0:256 / 256:512) so the dec matmuls can start after
only half the weight bytes have landed.

Softmax skips the max-subtraction: |scores| <= ||w2||_1 ~ 11, well inside
exp's fp32 range.
"""

import numpy as np

B, LIN, E, D, N = 32, 2048, 512, 512, 2
NCORES = 8
BPC = B // NCORES      # batches per core
P = 128                # SBUF partitions
ETILES = E // P        # 4
DTILES = D // P        # 4
ND = N * D             # 1024
KTILES = ND // P       # 8
LCHW = 512             # Lin chunk width (one PSUM bank of fp32)
LCH = LIN // LCHW      # 4

ENC_SCALE = 32.0       # enc pre-scale into fp8e4
W1E_SCALE = 8192.0     # W1_e pre-scale into fp8e4
INV_SCALE = 1.0 / (ENC_SCALE * W1E_SCALE)   # 2^-18, exact

# wmisc (bf16): dec-hidden columns + w2 columns
DH_LEN = KTILES * BPC          # 32: [k, b] -> d_flat[b, k*P+p]
W2_OFF = DH_LEN
W2_LEN = DTILES                # 4:  [a]    -> w2[a*P+p]
WMISC = DH_LEN + W2_LEN        # 36
DHALF = D // 2                 # 256
W1E_LEN = ETILES * D           # 2048: [e, d] -> W1_e[e*P+p, d] (fp8)

SCP = 3 * 32 + 1               # 97: score rows live at partitions {0,32,64,96}

TRACE = False
TRACE_KWARGS = {}
LAST_RESULT = None

_CACHE = {}


def _build():
    import concourse.bacc as bacc
    import concourse.mybir as mybir
    import concourse.tile as tile
    from concourse.bass import ts

    from concourse import bass_isa

    f32 = mybir.dt.float32
    bf16 = mybir.dt.bfloat16
    fp8 = mybir.dt.float8e4
    AF = mybir.ActivationFunctionType
    DR = mybir.MatmulPerfMode.DoubleRow

    nc = bacc.Bacc("TRN2", target_bir_lowering=False)

    encC_h = nc.dram_tensor(
        "encC", [BPC, LCH, P, ETILES, LCHW], fp8, kind="ExternalInput"
    )
    head8_h = nc.dram_tensor("head8", [P, 2 * W1E_LEN], fp8, kind="ExternalInput")
    wmisc_h = nc.dram_tensor("wmisc", [P, WMISC], bf16, kind="ExternalInput")
    # w1d0b: first half of W1_d (d 0:256) ++ b1 columns, all bf16
    w1d0b_h = nc.dram_tensor(
        "w1d0b", [P, KTILES * DHALF + DTILES], bf16, kind="ExternalInput"
    )
    w1d1_h = nc.dram_tensor("w1d1", [P, KTILES, DHALF], bf16, kind="ExternalInput")
    out_h = nc.dram_tensor("out", [BPC, LIN], f32, kind="ExternalOutput")

    with tile.TileContext(nc) as tc:
        with (
            tc.tile_pool(name="persist", bufs=1) as wp,
            tc.tile_pool(name="encp", bufs=BPC - 1) as encp,
            tc.tile_pool(name="attnp", bufs=20) as attnp,
            tc.tile_pool(name="smp", bufs=2) as smp,
            tc.tile_pool(name="mainps", bufs=3, space="PSUM") as mainps,
            tc.tile_pool(name="scpsp", bufs=1, space="PSUM") as scpsp,
            tc.tile_pool(name="decps", bufs=1, space="PSUM") as decps,
        ):
            # --- critical path: w1e + first enc chunk fused in ONE DMA ---
            head_sb = wp.tile([P, 2 * ETILES, LCHW], fp8, tag="head8")
            nc.sync.dma_start(
                out=head_sb, in_=head8_h.rearrange("p (e d) -> p e d", e=2 * ETILES)
            )
            w1e_sb = head_sb[:, 0:ETILES, :]

            # batch 0: chunk-granular tiles (fast first-compute); 1-3: one
            # batch tile + one DMA each
            enc_b0 = [
                encp.tile([P, ETILES, LCHW], fp8, tag="enc0", name=f"enc0l{lc}")
                for lc in range(1, LCH)
            ]
            enc_bt = [
                encp.tile(
                    [P, LCH, ETILES, LCHW], fp8, tag="encb", name=f"encb{b}"
                )
                for b in range(1, BPC)
            ]
            enc_tiles = [[head_sb[:, ETILES : 2 * ETILES, :]] + enc_b0] + [
                [enc_bt[b - 1][:, lc] for lc in range(LCH)] for b in range(1, BPC)
            ]

            w1d0b_sb = wp.tile([P, KTILES * DHALF + DTILES], bf16, tag="w1d0b")
            nc.sync.dma_start(out=w1d0b_sb, in_=w1d0b_h[:, :])
            w1d_sb = [
                w1d0b_sb[:, 0 : KTILES * DHALF].rearrange(
                    "p (k d) -> p k d", k=KTILES
                ),
                wp.tile([P, KTILES, DHALF], bf16, tag="w1d1", name="w1d1"),
            ]
            b1_bf = w1d0b_sb[:, KTILES * DHALF :]
            b1_sb = wp.tile([P, DTILES], f32, tag="b1f")
            nc.scalar.copy(out=b1_sb, in_=b1_bf)
            wmisc_sb = wp.tile([P, WMISC], bf16, tag="wmisc")
            nc.sync.dma_start(out=wmisc_sb, in_=wmisc_h[:, :])
            nc.sync.dma_start(out=w1d_sb[1], in_=w1d1_h[:, :, :])

            dh_sb = wmisc_sb[:, 0:DH_LEN].rearrange("p (k b) -> p k b", k=KTILES)
            w2_sb = wmisc_sb[:, W2_OFF : W2_OFF + W2_LEN]

            decb = wp.tile([P, DTILES, BPC], f32, tag="decb")

            # PE clock-gate warmup: the HAM throttles the PE to half clock
            # after ~3.4us idle, and the input DMA leaves the PE idle for
            # ~12us at kernel start.  A short stream of result-less matmuls
            # (no readers; WAW-serialized on one spare-shape dec tile) keeps
            # the PE active through the DMA wait so the first real matmuls
            # run at full clock.  Sized to drain before the first enc chunk
            # lands even when running throttled.
            warmsrc = wp.tile([P, P], bf16, tag="warmsrc")
            nc.vector.memset(warmsrc, 0.0)
            for w in range(28):
                wps = decps.tile([P, BPC], f32, tag="d", name=f"warm{w}")
                nc.tensor.matmul(
                    out=wps, lhsT=warmsrc, rhs=warmsrc[:, 0:BPC]
                )

            def emit_dec(js):
                # dec_projT + b1 bias columns: [p, dtile, batch]
                for j in js:
                    dps = decps.tile([P, BPC], f32, tag="d", name=f"decps{j}")
                    for k in range(KTILES):
                        nc.tensor.matmul(
                            out=dps,
                            lhsT=w1d_sb[j // 2][:, k, ts(j % 2, P)],
                            rhs=dh_sb[:, k, :],
                            start=(k == 0),
                            stop=(k == KTILES - 1),
                        )
                    nc.vector.tensor_scalar_add(
                        out=decb[:, j, :], in0=dps, scalar1=b1_sb[:, j : j + 1]
                    )

            # remaining enc DMAs, in consumption order (Sync trigger pacing
            # naturally prioritizes earlier data)
            for lc in range(1, LCH):
                nc.sync.dma_start(out=enc_b0[lc - 1], in_=encC_h[0, lc])
            for b in range(1, BPC):
                nc.sync.dma_start(
                    out=enc_bt[b - 1],
                    in_=encC_h[b].rearrange("l p e w -> p l e w"),
                )

            # --- main loop over 2-chunk slots ---
            # Each slot computes TWO Lin-chunks: the four j-groups land in
            # [P, 2, LCHW] double-bank PSUM tiles (ring of 3) so ONE tanh
            # activation covers both chunks of a j (same per-partition
            # dec-bias), halving the ACT per-op overhead count.
            # Scores for batch b are emitted after batch b+1's first slot
            # as column-tiled quads: the 4 chunks' M=1 matmuls target
            # distinct 32-column groups (partitions 0/32/64/96), so the PE
            # array runs each quad's 4 streams concurrently.
            slots = [(b, h) for b in range(BPC) for h in range(LCH // 2)]
            scs_tiles = {}
            attn_tiles = {}
            sume_tiles = {}

            def emit_scores_batch(b, js=tuple(range(DTILES))):
                sc = scs_tiles[b]
                for j in js:
                    for lc in range(LCH):
                        at = attn_tiles[(b, lc // 2)][j]
                        nc.tensor.matmul(
                            out=sc[32 * lc : 32 * lc + 1, :],
                            lhsT=w2_sb[:, j : j + 1],
                            rhs=at[:, lc % 2, :],
                            start=(j == 0),
                            stop=(j == DTILES - 1),
                            tile_position=(0, 32 * lc),
                        )
                if js[-1] == DTILES - 1:
                    for h in range(LCH // 2):
                        attn_tiles.pop((b, h))

            def emit_exp(b):
                # one Exp for all 4 chunks (rows 0/32/64/96 + zeroed filler)
                erow = smp.tile([SCP, LCHW], f32, tag="erow", name=f"erow{b}")
                sume = smp.tile([SCP, 1], f32, tag="sume", name=f"sume{b}")
                nc.scalar.activation(
                    out=erow, in_=scs_tiles[b], func=AF.Exp, bias=0.0, scale=1.0,
                    accum_out=sume,
                )
                sume_tiles[b] = (erow, sume)

            def emit_tail2(b):
                # all-partition sum of per-chunk exp sums -> 1/sum -> scale
                erow, sume = sume_tiles.pop(b)
                scs_tiles.pop(b)
                sumall = smp.tile([SCP, 1], f32, tag="sumall", name=f"sumall{b}")
                nc.gpsimd.partition_all_reduce(
                    sumall, sume, SCP, bass_isa.ReduceOp.add
                )
                rinv97 = smp.tile([SCP, 1], f32, tag="rinv97", name=f"rinv97{b}")
                nc.vector.reciprocal(out=rinv97, in_=sumall)
                orow = smp.tile([SCP, LCHW], f32, tag="orow", name=f"orow{b}")
                nc.vector.tensor_scalar_mul(out=orow, in0=erow, scalar1=rinv97)
                nc.sync.dma_start(
                    out=out_h[b : b + 1, :].rearrange("o (c w) -> o c w", c=LCH),
                    in_=orow[0 : 3 * 32 + 1 : 32, :],
                )

            for i, (b, h) in enumerate(slots):
                ca, cb = 2 * h, 2 * h + 1
                mpss = []
                for j in range(DTILES):
                    mps = mainps.tile(
                        [P, 2, LCHW], f32, tag="m", name=f"mps_b{b}h{h}j{j}"
                    )
                    for c in (0, 1):
                        for t in range(ETILES // 2):
                            nc.tensor.matmul(
                                out=mps[:, c, :],
                                lhsT=w1e_sb[:, 2 * t : 2 * t + 2, ts(j, P)],
                                rhs=enc_tiles[b][ca + c][:, 2 * t : 2 * t + 2, :],
                                start=(t == 0),
                                stop=(t == ETILES // 2 - 1),
                                perf_mode=DR,
                            )
                    mpss.append(mps)

                if i == 0:
                    emit_dec((0, 1, 2, 3))
                if h == 0 and b >= 1:
                    emit_scores_batch(b - 1)
                if h == 1:
                    if b >= 1:
                        emit_tail2(b - 1)
                    # score bank for batch b; its only gen-(b-1) reader is
                    # exp(b-1), one slot back — must precede scores(b) quads
                    sc = scpsp.tile([SCP, LCHW], f32, tag="sc", name=f"sc{b}")
                    scs_tiles[b] = sc
                    nc.vector.memset(sc, -100.0)

                attns = []
                for j in range(DTILES):
                    at = attnp.tile(
                        [P, 2, LCHW], bf16, tag="attn", name=f"attn_b{b}h{h}j{j}"
                    )
                    nc.scalar.activation(
                        out=at,
                        in_=mpss[j],
                        func=AF.Tanh,
                        bias=decb[:, j, b : b + 1],
                        scale=INV_SCALE,
                    )
                    attns.append(at)
                attn_tiles[(b, h)] = attns
                if h == 0 and b >= 1:
                    # emitted after this slot's tanhs so the in-order ACT
                    # queue never parks on the Exp while tanh work is ready
                    emit_exp(b - 1)
                if i == len(slots) - 1:
                    # last batch: j0/j1 quads run as soon as this slot's
                    # early tanhs land, shortening the tail
                    emit_scores_batch(b, (0, 1))

            b_last = BPC - 1
            emit_scores_batch(b_last, (2, 3))
            emit_exp(b_last)
            emit_tail2(b_last)
    nc.compile()
    return nc


def _prep_in_maps(d_hidden, encoder_outputs, W1, b1, w2):
    import ml_dtypes

    bf = ml_dtypes.bfloat16
    f8 = ml_dtypes.float8_e4m3
    d_hidden = np.ascontiguousarray(np.asarray(d_hidden), dtype=np.float32)
    encoder_outputs = np.asarray(encoder_outputs)
    W1 = np.ascontiguousarray(np.asarray(W1), dtype=np.float32)
    b1 = np.ascontiguousarray(np.asarray(b1), dtype=np.float32)
    w2 = np.ascontiguousarray(np.asarray(w2), dtype=np.float32)

    W1d, W1e = W1[:ND], W1[ND:]
    w1e8 = np.ascontiguousarray(
        (W1e * W1E_SCALE)
        .reshape(ETILES, P, D)
        .transpose(1, 0, 2)
        .reshape(P, W1E_LEN)
        .astype(f8)
    )
    w1dk = W1d.reshape(KTILES, P, D).transpose(1, 0, 2).astype(bf)  # [P, k, D]
    w1d0b = np.concatenate(
        [
            w1dk[:, :, :DHALF].reshape(P, KTILES * DHALF),
            b1.reshape(DTILES, P).T.astype(bf),
        ],
        axis=1,
    )
    w1d1 = np.ascontiguousarray(w1dk[:, :, DHALF:])

    in_maps = []
    for c in range(NCORES):
        bs = slice(c * BPC, (c + 1) * BPC)
        encT = (
            np.asarray(encoder_outputs[bs], dtype=np.float32).transpose(0, 2, 1)
            * ENC_SCALE
        )  # [BPC, E, LIN] scaled
        encC = np.ascontiguousarray(
            encT.reshape(BPC, ETILES, P, LCH, LCHW)
            .transpose(0, 3, 2, 1, 4)
            .astype(f8)
        )
        dhT = np.ascontiguousarray(d_hidden[bs].reshape(BPC, ND).T)  # [ND, BPC]
        wmisc = np.zeros((P, WMISC), dtype=bf)
        wmisc[:, 0:DH_LEN] = (
            dhT.reshape(KTILES, P, BPC).transpose(1, 0, 2).reshape(P, DH_LEN)
            .astype(bf)
        )
        wmisc[:, W2_OFF : W2_OFF + W2_LEN] = w2.reshape(DTILES, P).T.astype(bf)
        head8 = np.concatenate(
            [w1e8, encC[0, 0].reshape(P, ETILES * LCHW)], axis=1
        )
        in_maps.append(
            {
                "encC": encC,
                "head8": head8,
                "wmisc": wmisc,
                "w1d0b": w1d0b,
                "w1d1": w1d1,
            }
        )
    return in_maps


def kernel(d_hidden, encoder_outputs, W1, b1, w2):
    global LAST_RESULT
    from concourse import bass_utils

    if "nc" not in _CACHE:
        _CACHE["nc"] = _build()
    nc = _CACHE["nc"]

    in_maps = _prep_in_maps(d_hidden, encoder_outputs, W1, b1, w2)
    res = bass_utils.run_bass_kernel_spmd(
        nc,
        in_maps,
        core_ids=list(range(NCORES)),
        trace=TRACE,
        **TRACE_KWARGS,
    )
    LAST_RESULT = res
    return np.concatenate([r["out"] for r in res.results], axis=0)



# revision 8
# speedup vs baseline: 1.0025x; 1.0025x over previous
"""Bass/Trainium2 kernel for nn_Attention_42305427865835.

Computes, for d_hidden [B,N,D], encoder_outputs [B,Lin,E], W1 [E+N*D, D],
b1 [D], w2 [D]:
    dec_proj = d_flat @ W1[:N*D] + b1                    # [B, D]
    enc_proj = enc @ W1[N*D:]                            # [B, Lin, E->D]
    scores   = tanh(enc_proj + dec_proj[:,None,:]) @ w2  # [B, Lin]
    out      = softmax(scores, axis=-1)
sharded data-parallel over batch, 4 batches per core on 8 cores.

Device-side layout is transposed ("T layout": D/E on partitions, Lin on the
free axis) so the contraction over E maps onto the PE array and the
dec_proj/b1 bias-add rides the ScalarE activation's per-partition bias.

The enc matmul (the dominant FLOPs) runs in fp8e4 with
MatmulPerfMode.DoubleRow: host pre-scales enc by 32 and W1_e by 8192
(keeping both inside fp8e4's +-240 range), packs the contraction as
[P, etile, free] so an e-tile PAIR is one K=256 DoubleRow matmul, and the
tanh activation's scale=2^-18 undoes the scaling exactly.  The score matmul
stays bf16 (fp8 there would blow the error budget).  Simulated end-to-end
absmax-relative error 1.83e-2 (gate 2e-2); the same simulator matches the
bf16 baseline's hardware error to 3 digits.

Softmax: scores for the 4 Lin-chunks of a batch land on PSUM partitions
{0,32,64,96} of one bank (tile_position picks the column group), so ONE Exp
activation covers the whole batch and its accum_out gives per-chunk sums.
The bank is memset to -100 first so unused partitions exp to 0, making the
ones-vector partition-sum matmul exact; gpsimd.partition_broadcast spreads
1/sum back across partitions for the final scale.  The partition-sum matmul
reuses element [0,0] of the score bank (no spare PSUM bank exists), and the
tail is pipelined across two chunk slots so the PE never waits on the Exp.

Score matmuls are emitted one chunk behind the enc matmuls so the PE queue
never head-blocks on the tanh that produces their input.  W1_d comes in two
half-tensors (d columns 0:256 / 256:512) so the dec matmuls can start after
only half the weight bytes have landed.

All enc chunks transfer as individual [P, 2048-byte] DMAs in consumption
order (whole-batch DMAs only signal on the last byte, which starved the PE
~3.4us at each batch handoff and re-throttled the HAM clock gate); the dec
matmuls weave between slot-0 enc chunk groups so decb[j] lands just before
tanh j first needs it and the PE has no idle window the HAM could see.

Softmax skips the max-subtraction: |scores| <= ||w2||_1 ~ 11, well inside
exp's fp32 range.
"""

import numpy as np

B, LIN, E, D, N = 32, 2048, 512, 512, 2
NCORES = 8
BPC = B // NCORES      # batches per core
P = 128                # SBUF partitions
ETILES = E // P        # 4
DTILES = D // P        # 4
ND = N * D             # 1024
KTILES = ND // P       # 8
LCHW = 512             # Lin chunk width (one PSUM bank of fp32)
LCH = LIN // LCHW      # 4

ENC_SCALE = 32.0       # enc pre-scale into fp8e4
W1E_SCALE = 8192.0     # W1_e pre-scale into fp8e4
INV_SCALE = 1.0 / (ENC_SCALE * W1E_SCALE)   # 2^-18, exact

# wmisc (bf16): dec-hidden columns + w2 columns
DH_LEN = KTILES * BPC          # 32: [k, b] -> d_flat[b, k*P+p]
W2_OFF = DH_LEN
W2_LEN = DTILES                # 4:  [a]    -> w2[a*P+p]
WMISC = DH_LEN + W2_LEN        # 36
DHALF = D // 2                 # 256
W1E_LEN = ETILES * D           # 2048: [e, d] -> W1_e[e*P+p, d] (fp8)

SCP = 3 * 32 + 1               # 97: score rows live at partitions {0,32,64,96}

TRACE = False
TRACE_KWARGS = {}
LAST_RESULT = None

_CACHE = {}


def _build():
    import concourse.bacc as bacc
    import concourse.mybir as mybir
    import concourse.tile as tile
    from concourse.bass import ts

    from concourse import bass_isa

    f32 = mybir.dt.float32
    bf16 = mybir.dt.bfloat16
    fp8 = mybir.dt.float8e4
    AF = mybir.ActivationFunctionType
    DR = mybir.MatmulPerfMode.DoubleRow

    nc = bacc.Bacc("TRN2", target_bir_lowering=False)

    encC_h = nc.dram_tensor(
        "encC", [BPC, LCH, P, ETILES, LCHW], fp8, kind="ExternalInput"
    )
    head8_h = nc.dram_tensor("head8", [P, 2 * W1E_LEN], fp8, kind="ExternalInput")
    wmisc_h = nc.dram_tensor("wmisc", [P, WMISC], bf16, kind="ExternalInput")
    # w1d0b: first half of W1_d (d 0:256) ++ b1 columns, all bf16
    w1d0b_h = nc.dram_tensor(
        "w1d0b", [P, KTILES * DHALF + DTILES], bf16, kind="ExternalInput"
    )
    w1d1_h = nc.dram_tensor("w1d1", [P, KTILES, DHALF], bf16, kind="ExternalInput")
    out_h = nc.dram_tensor("out", [BPC, LIN], f32, kind="ExternalOutput")

    with tile.TileContext(nc) as tc:
        with (
            tc.tile_pool(name="persist", bufs=1) as wp,
            tc.tile_pool(name="encp", bufs=LCH * BPC - 1) as encp,
            tc.tile_pool(name="attnp", bufs=20) as attnp,
            tc.tile_pool(name="smp", bufs=2) as smp,
            tc.tile_pool(name="mainps", bufs=3, space="PSUM") as mainps,
            tc.tile_pool(name="scpsp", bufs=1, space="PSUM") as scpsp,
            tc.tile_pool(name="decps", bufs=1, space="PSUM") as decps,
        ):
            # --- critical path: w1e + first enc chunk fused in ONE DMA ---
            head_sb = wp.tile([P, 2 * ETILES, LCHW], fp8, tag="head8")
            nc.sync.dma_start(
                out=head_sb, in_=head8_h.rearrange("p (e d) -> p e d", e=2 * ETILES)
            )
            w1e_sb = head_sb[:, 0:ETILES, :]

            # all enc chunks land as individual [P, ETILES, LCHW] DMAs so
            # compute unblocks at chunk granularity (a whole-batch DMA only
            # signals when its last byte lands, which starved the PE for
            # ~3.4us at the batch 0 -> 1 handoff and re-throttled the HAM)
            enc_ch = [
                encp.tile(
                    [P, ETILES, LCHW], fp8, tag="enc", name=f"enc{b}c{lc}"
                )
                for b in range(BPC)
                for lc in range(LCH)
                if (b, lc) != (0, 0)
            ]
            enc_tiles = [[head_sb[:, ETILES : 2 * ETILES, :]] + enc_ch[0:3]] + [
                enc_ch[4 * b - 1 : 4 * b + 3] for b in range(1, BPC)
            ]

            w1d0b_sb = wp.tile([P, KTILES * DHALF + DTILES], bf16, tag="w1d0b")
            w1d_sb = [
                w1d0b_sb[:, 0 : KTILES * DHALF].rearrange(
                    "p (k d) -> p k d", k=KTILES
                ),
                wp.tile([P, KTILES, DHALF], bf16, tag="w1d1", name="w1d1"),
            ]
            b1_bf = w1d0b_sb[:, KTILES * DHALF :]
            b1_sb = wp.tile([P, DTILES], f32, tag="b1f")
            wmisc_sb = wp.tile([P, WMISC], bf16, tag="wmisc")

            # trigger order == consumption order: wmisc + w1d feed the dec
            # matmuls woven into slot 0, so they go right after the head;
            # enc chunks follow in slot order.
            nc.sync.dma_start(out=wmisc_sb, in_=wmisc_h[:, :])
            nc.sync.dma_start(out=w1d0b_sb, in_=w1d0b_h[:, :])
            nc.sync.dma_start(out=enc_ch[0], in_=encC_h[0, 1])
            nc.sync.dma_start(out=enc_ch[1], in_=encC_h[0, 2])
            nc.sync.dma_start(out=w1d_sb[1], in_=w1d1_h[:, :, :])
            nc.sync.dma_start(out=enc_ch[2], in_=encC_h[0, 3])
            for b in range(1, BPC):
                for lc in range(LCH):
                    nc.sync.dma_start(
                        out=enc_ch[4 * b + lc - 1], in_=encC_h[b, lc]
                    )

            nc.vector.tensor_copy(out=b1_sb, in_=b1_bf)

            dh_sb = wmisc_sb[:, 0:DH_LEN].rearrange("p (k b) -> p k b", k=KTILES)
            w2_sb = wmisc_sb[:, W2_OFF : W2_OFF + W2_LEN]

            decb = wp.tile([P, DTILES, BPC], f32, tag="decb")

            # PE clock-gate warmup: the HAM throttles the PE to half clock
            # after ~3.4us idle, and the runtime preamble + input DMA leave
            # the PE idle for ~9.5us at kernel start.  A short stream of
            # result-less matmuls (no readers; WAW-serialized on one spare
            # tile) keeps the PE active through the DMA wait so the HAM
            # un-throttles shortly after real matmuls begin.  Sized to drain
            # right as the first enc chunk lands (~9.6us); oversizing this
            # delays all real work (PE queue is FIFO).
            # memset on GpSimd: every engine's queue preamble ends ~5.1us,
            # and GpSimd is otherwise idle until the first softmax tail
            # (~21us), so the PE's first warmup matmul isn't queued behind
            # the DVE's busier early queue.
            warmsrc = wp.tile([P, P], bf16, tag="warmsrc")
            nc.gpsimd.memset(warmsrc, 0.0)
            for w in range(18):
                wps = decps.tile([P, BPC], f32, tag="d", name=f"warm{w}")
                nc.tensor.matmul(
                    out=wps, lhsT=warmsrc, rhs=warmsrc[:, 0:BPC]
                )

            def emit_dec(js):
                # dec_projT + b1 bias columns: [p, dtile, batch]
                for j in js:
                    dps = decps.tile([P, BPC], f32, tag="d", name=f"decps{j}")
                    for k in range(KTILES):
                        nc.tensor.matmul(
                            out=dps,
                            lhsT=w1d_sb[j // 2][:, k, ts(j % 2, P)],
                            rhs=dh_sb[:, k, :],
                            start=(k == 0),
                            stop=(k == KTILES - 1),
                        )
                    nc.vector.tensor_scalar_add(
                        out=decb[:, j, :], in0=dps, scalar1=b1_sb[:, j : j + 1]
                    )

            # --- main loop over 2-chunk slots ---
            # Each slot computes TWO Lin-chunks: the four j-groups land in
            # [P, 2, LCHW] double-bank PSUM tiles (ring of 3) so ONE tanh
            # activation covers both chunks of a j (same per-partition
            # dec-bias), halving the ACT per-op overhead count.
            # Scores for batch b are emitted after batch b+1's first slot
            # as column-tiled quads: the 4 chunks' M=1 matmuls target
            # distinct 32-column groups (partitions 0/32/64/96), so the PE
            # array runs each quad's 4 streams concurrently.
            slots = [(b, h) for b in range(BPC) for h in range(LCH // 2)]
            scs_tiles = {}
            attn_tiles = {}
            sume_tiles = {}

            def emit_scores_batch(b, js=tuple(range(DTILES))):
                sc = scs_tiles[b]
                for j in js:
                    for lc in range(LCH):
                        at = attn_tiles[(b, lc // 2)][j]
                        nc.tensor.matmul(
                            out=sc[32 * lc : 32 * lc + 1, :],
                            lhsT=w2_sb[:, j : j + 1],
                            rhs=at[:, lc % 2, :],
                            start=(j == 0),
                            stop=(j == DTILES - 1),
                            tile_position=(0, 32 * lc),
                        )
                if js[-1] == DTILES - 1:
                    for h in range(LCH // 2):
                        attn_tiles.pop((b, h))

            def emit_exp(b):
                # one Exp for all 4 chunks (rows 0/32/64/96 + zeroed filler)
                erow = smp.tile([SCP, LCHW], f32, tag="erow", name=f"erow{b}")
                sume = smp.tile([SCP, 1], f32, tag="sume", name=f"sume{b}")
                nc.scalar.activation(
                    out=erow, in_=scs_tiles[b], func=AF.Exp, bias=0.0, scale=1.0,
                    accum_out=sume,
                )
                sume_tiles[b] = (erow, sume)

            def emit_tail2(b):
                # all-partition sum of per-chunk exp sums -> 1/sum -> scale
                erow, sume = sume_tiles.pop(b)
                scs_tiles.pop(b)
                sumall = smp.tile([SCP, 1], f32, tag="sumall", name=f"sumall{b}")
                nc.gpsimd.partition_all_reduce(
                    sumall, sume, SCP, bass_isa.ReduceOp.add
                )
                rinv97 = smp.tile([SCP, 1], f32, tag="rinv97", name=f"rinv97{b}")
                nc.vector.reciprocal(out=rinv97, in_=sumall)
                orow = smp.tile([SCP, LCHW], f32, tag="orow", name=f"orow{b}")
                nc.vector.tensor_scalar_mul(out=orow, in0=erow, scalar1=rinv97)
                nc.sync.dma_start(
                    out=out_h[b : b + 1, :].rearrange("o (c w) -> o c w", c=LCH),
                    in_=orow[0 : 3 * 32 + 1 : 32, :],
                )

            for i, (b, h) in enumerate(slots):
                ca, cb = 2 * h, 2 * h + 1
                mpss = []
                for j in range(DTILES):
                    mps = mainps.tile(
                        [P, 2, LCHW], f32, tag="m", name=f"mps_b{b}h{h}j{j}"
                    )
                    for c in (0, 1):
                        for t in range(ETILES // 2):
                            nc.tensor.matmul(
                                out=mps[:, c, :],
                                lhsT=w1e_sb[:, 2 * t : 2 * t + 2, ts(j, P)],
                                rhs=enc_tiles[b][ca + c][:, 2 * t : 2 * t + 2, :],
                                start=(t == 0),
                                stop=(t == ETILES // 2 - 1),
                                perf_mode=DR,
                            )
                    mpss.append(mps)
                    if i == 0:
                        # weave dec j-groups between slot-0 enc chunk groups:
                        # decb[j] is ready right before tanh j needs it, the
                        # decps WAW reuse stall hides under enc matmuls, and
                        # the PE stays dense enough that the HAM never sees
                        # an idle MID window here
                        emit_dec((j,))
                if h == 0 and b >= 1:
                    emit_scores_batch(b - 1)
                if h == 1:
                    if b >= 1:
                        emit_tail2(b - 1)
                    # score bank for batch b; its only gen-(b-1) reader is
                    # exp(b-1), one slot back — must precede scores(b) quads
                    sc = scpsp.tile([SCP, LCHW], f32, tag="sc", name=f"sc{b}")
                    scs_tiles[b] = sc
                    nc.vector.memset(sc, -100.0)

                attns = []
                for j in range(DTILES):
                    at = attnp.tile(
                        [P, 2, LCHW], bf16, tag="attn", name=f"attn_b{b}h{h}j{j}"
                    )
                    nc.scalar.activation(
                        out=at,
                        in_=mpss[j],
                        func=AF.Tanh,
                        bias=decb[:, j, b : b + 1],
                        scale=INV_SCALE,
                    )
                    attns.append(at)
                attn_tiles[(b, h)] = attns
                if h == 0 and b >= 1:
                    # emitted after this slot's tanhs so the in-order ACT
                    # queue never parks on the Exp while tanh work is ready
                    emit_exp(b - 1)
                if i == len(slots) - 1:
                    # last batch: j0/j1 quads run as soon as this slot's
                    # early tanhs land, shortening the tail
                    emit_scores_batch(b, (0, 1))

            b_last = BPC - 1
            emit_scores_batch(b_last, (2, 3))
            emit_exp(b_last)
            emit_tail2(b_last)
    nc.compile()
    return nc


def _prep_in_maps(d_hidden, encoder_outputs, W1, b1, w2):
    import ml_dtypes

    bf = ml_dtypes.bfloat16
    f8 = ml_dtypes.float8_e4m3
    d_hidden = np.ascontiguousarray(np.asarray(d_hidden), dtype=np.float32)
    encoder_outputs = np.asarray(encoder_outputs)
    W1 = np.ascontiguousarray(np.asarray(W1), dtype=np.float32)
    b1 = np.ascontiguousarray(np.asarray(b1), dtype=np.float32)
    w2 = np.ascontiguousarray(np.asarray(w2), dtype=np.float32)

    W1d, W1e = W1[:ND], W1[ND:]
    w1e8 = np.ascontiguousarray(
        (W1e * W1E_SCALE)
        .reshape(ETILES, P, D)
        .transpose(1, 0, 2)
        .reshape(P, W1E_LEN)
        .astype(f8)
    )
    w1dk = W1d.reshape(KTILES, P, D).transpose(1, 0, 2).astype(bf)  # [P, k, D]
    w1d0b = np.concatenate(
        [
            w1dk[:, :, :DHALF].reshape(P, KTILES * DHALF),
            b1.reshape(DTILES, P).T.astype(bf),
        ],
        axis=1,
    )
    w1d1 = np.ascontiguousarray(w1dk[:, :, DHALF:])

    in_maps = []
    for c in range(NCORES):
        bs = slice(c * BPC, (c + 1) * BPC)
        encT = (
            np.asarray(encoder_outputs[bs], dtype=np.float32).transpose(0, 2, 1)
            * ENC_SCALE
        )  # [BPC, E, LIN] scaled
        encC = np.ascontiguousarray(
            encT.reshape(BPC, ETILES, P, LCH, LCHW)
            .transpose(0, 3, 2, 1, 4)
            .astype(f8)
        )
        dhT = np.ascontiguousarray(d_hidden[bs].reshape(BPC, ND).T)  # [ND, BPC]
        wmisc = np.zeros((P, WMISC), dtype=bf)
        wmisc[:, 0:DH_LEN] = (
            dhT.reshape(KTILES, P, BPC).transpose(1, 0, 2).reshape(P, DH_LEN)
            .astype(bf)
        )
        wmisc[:, W2_OFF : W2_OFF + W2_LEN] = w2.reshape(DTILES, P).T.astype(bf)
        head8 = np.concatenate(
            [w1e8, encC[0, 0].reshape(P, ETILES * LCHW)], axis=1
        )
        in_maps.append(
            {
                "encC": encC,
                "head8": head8,
                "wmisc": wmisc,
                "w1d0b": w1d0b,
                "w1d1": w1d1,
            }
        )
    return in_maps


def kernel(d_hidden, encoder_outputs, W1, b1, w2):
    global LAST_RESULT
    from concourse import bass_utils

    if "nc" not in _CACHE:
        _CACHE["nc"] = _build()
    nc = _CACHE["nc"]

    in_maps = _prep_in_maps(d_hidden, encoder_outputs, W1, b1, w2)
    res = bass_utils.run_bass_kernel_spmd(
        nc,
        in_maps,
        core_ids=list(range(NCORES)),
        trace=TRACE,
        **TRACE_KWARGS,
    )
    LAST_RESULT = res
    return np.concatenate([r["out"] for r in res.results], axis=0)



# revision 16
# speedup vs baseline: 1.0332x; 1.0306x over previous
"""Bass/Trainium2 kernel for nn_Attention_42305427865835.

Computes, for d_hidden [B,N,D], encoder_outputs [B,Lin,E], W1 [E+N*D, D],
b1 [D], w2 [D]:
    dec_proj = d_flat @ W1[:N*D] + b1                    # [B, D]
    enc_proj = enc @ W1[N*D:]                            # [B, Lin, E->D]
    scores   = tanh(enc_proj + dec_proj[:,None,:]) @ w2  # [B, Lin]
    out      = softmax(scores, axis=-1)
sharded data-parallel over batch, 4 batches per core on 8 cores.

Device-side layout is transposed ("T layout": D/E on partitions, Lin on the
free axis) so the contraction over E maps onto the PE array and the
dec_proj/b1 bias-add rides the ScalarE activation's per-partition bias.

The enc matmul (the dominant FLOPs) runs in fp8e4 with
MatmulPerfMode.DoubleRow: host pre-scales enc by 32 and W1_e by 8192
(keeping both inside fp8e4's +-240 range), packs the contraction as
[P, etile, free] so an e-tile PAIR is one K=256 DoubleRow matmul, and the
tanh activation's scale=2^-18 undoes the scaling exactly.  The score matmul
stays bf16 (fp8 there would blow the error budget).  Simulated end-to-end
absmax-relative error 1.83e-2 (gate 2e-2); the same simulator matches the
bf16 baseline's hardware error to 3 digits.

Softmax: scores for the 4 Lin-chunks of a batch land on PSUM partitions
{0,32,64,96} of one bank (tile_position picks the column group), so ONE Exp
activation covers the whole batch and its accum_out gives per-chunk sums.
The bank is memset to -100 first so unused partitions exp to 0, making the
ones-vector partition-sum matmul exact; gpsimd.partition_broadcast spreads
1/sum back across partitions for the final scale.  The partition-sum matmul
reuses element [0,0] of the score bank (no spare PSUM bank exists), and the
tail is pipelined across two chunk slots so the PE never waits on the Exp.

Score matmuls are emitted one chunk behind the enc matmuls so the PE queue
never head-blocks on the tanh that produces their input.  W1_d comes in two
half-tensors (d columns 0:256 / 256:512) so the dec matmuls can start after
only half the weight bytes have landed.

All enc chunks transfer as individual [P, 2048-byte] DMAs in consumption
order (whole-batch DMAs only signal on the last byte, which starved the PE
~3.4us at each batch handoff and re-throttled the HAM clock gate); the dec
matmuls weave between slot-0 enc chunk groups so decb[j] lands just before
tanh j first needs it and the PE has no idle window the HAM could see.

Softmax skips the max-subtraction: |scores| <= ||w2||_1 ~ 11, well inside
exp's fp32 range.
"""

import numpy as np

B, LIN, E, D, N = 32, 2048, 512, 512, 2
NCORES = 8
BPC = B // NCORES      # batches per core
P = 128                # SBUF partitions
ETILES = E // P        # 4
DTILES = D // P        # 4
ND = N * D             # 1024
KTILES = ND // P       # 8
LCHW = 512             # Lin chunk width (one PSUM bank of fp32)
LCH = LIN // LCHW      # 4

ENC_SCALE = 32.0       # enc pre-scale into fp8e4
W1E_SCALE = 8192.0     # W1_e pre-scale into fp8e4
INV_SCALE = 1.0 / (ENC_SCALE * W1E_SCALE)   # 2^-18, exact

# wmisc (bf16): dec-hidden columns + w2 columns
DH_LEN = KTILES * BPC          # 32: [k, b] -> d_flat[b, k*P+p]
W2_OFF = DH_LEN
W2_LEN = DTILES                # 4:  [a]    -> w2[a*P+p]
WMISC = DH_LEN + W2_LEN        # 36
DHALF = D // 2                 # 256
W1E_LEN = ETILES * D           # 2048: [e, d] -> W1_e[e*P+p, d] (fp8)

SCP = 3 * 32 + 1               # 97: score rows live at partitions {0,32,64,96}

TRACE = False
TRACE_KWARGS = {}
LAST_RESULT = None

_CACHE = {}


def _build():
    import concourse.bacc as bacc
    import concourse.mybir as mybir
    import concourse.tile as tile
    from concourse.bass import ts

    from concourse import bass_isa

    f32 = mybir.dt.float32
    bf16 = mybir.dt.bfloat16
    fp8 = mybir.dt.float8e4
    AF = mybir.ActivationFunctionType
    DR = mybir.MatmulPerfMode.DoubleRow

    nc = bacc.Bacc("TRN2", target_bir_lowering=False)

    encC_h = nc.dram_tensor(
        "encC", [BPC, LCH, P, ETILES, LCHW], fp8, kind="ExternalInput"
    )
    head8_h = nc.dram_tensor("head8", [P, 2 * W1E_LEN], fp8, kind="ExternalInput")
    wmisc_h = nc.dram_tensor("wmisc", [P, WMISC], bf16, kind="ExternalInput")
    # w1d0b: W1_d columns for dec j0 (d 0:128) ++ b1 columns, all bf16
    w1d0b_h = nc.dram_tensor(
        "w1d0b", [P, KTILES * P + DTILES], bf16, kind="ExternalInput"
    )
    # w1dj1: W1_d columns for dec j1 (d 128:256)
    w1dj1_h = nc.dram_tensor("w1dj1", [P, KTILES, P], bf16, kind="ExternalInput")
    w1d1_h = nc.dram_tensor("w1d1", [P, KTILES, DHALF], bf16, kind="ExternalInput")
    out_h = nc.dram_tensor("out", [BPC, LIN], f32, kind="ExternalOutput")

    with tile.TileContext(nc) as tc:
        with (
            tc.tile_pool(name="persist", bufs=1) as wp,
            tc.tile_pool(name="encp", bufs=LCH * BPC - 1) as encp,
            tc.tile_pool(name="attnp", bufs=20) as attnp,
            tc.tile_pool(name="smp", bufs=2) as smp,
            tc.tile_pool(name="mainps", bufs=3, space="PSUM") as mainps,
            tc.tile_pool(name="scpsp", bufs=1, space="PSUM") as scpsp,
            tc.tile_pool(name="decps", bufs=1, space="PSUM") as decps,
        ):
            # --- critical path: w1e + first enc chunk fused in ONE DMA ---
            head_sb = wp.tile([P, 2 * ETILES, LCHW], fp8, tag="head8")
            nc.sync.dma_start(
                out=head_sb, in_=head8_h.rearrange("p (e d) -> p e d", e=2 * ETILES)
            )
            w1e_sb = head_sb[:, 0:ETILES, :]

            # all enc chunks land as individual [P, ETILES, LCHW] DMAs so
            # compute unblocks at chunk granularity (a whole-batch DMA only
            # signals when its last byte lands, which starved the PE for
            # ~3.4us at the batch 0 -> 1 handoff and re-throttled the HAM)
            enc_ch = [
                encp.tile(
                    [P, ETILES, LCHW], fp8, tag="enc", name=f"enc{b}c{lc}"
                )
                for b in range(BPC)
                for lc in range(LCH)
                if (b, lc) != (0, 0)
            ]
            enc_tiles = [[head_sb[:, ETILES : 2 * ETILES, :]] + enc_ch[0:3]] + [
                enc_ch[4 * b - 1 : 4 * b + 3] for b in range(1, BPC)
            ]

            w1d0b_sb = wp.tile([P, KTILES * P + DTILES], bf16, tag="w1d0b")
            w1dj1_sb = wp.tile([P, KTILES, P], bf16, tag="w1dj1", name="w1dj1")
            w1d1_sb = wp.tile([P, KTILES, DHALF], bf16, tag="w1d1", name="w1d1")
            w1dj0 = w1d0b_sb[:, 0 : KTILES * P].rearrange("p (k d) -> p k d", k=KTILES)

            def w1d_j(j, k):  # lhsT [P, 128] for dec j-tile, k-tile
                if j == 0:
                    return w1dj0[:, k]
                if j == 1:
                    return w1dj1_sb[:, k]
                return w1d1_sb[:, k, ts(j - 2, P)]

            b1_bf = w1d0b_sb[:, KTILES * P :]
            b1_sb = wp.tile([P, DTILES], f32, tag="b1f")
            wmisc_sb = wp.tile([P, WMISC], bf16, tag="wmisc")

            # trigger order == consumption order, with the dec weights split
            # per j-tile so each piece is small enough not to starve the
            # head/chunk stream it shares HBM with: each dec j-group's
            # weights land just before the woven dec matmuls need them.
            nc.sync.dma_start(out=wmisc_sb, in_=wmisc_h[:, :])
            nc.sync.dma_start(out=w1d0b_sb, in_=w1d0b_h[:, :])
            nc.sync.dma_start(out=enc_ch[0], in_=encC_h[0, 1])
            nc.sync.dma_start(out=w1dj1_sb, in_=w1dj1_h[:, :, :])
            nc.sync.dma_start(out=enc_ch[1], in_=encC_h[0, 2])
            nc.sync.dma_start(out=w1d1_sb, in_=w1d1_h[:, :, :])
            nc.sync.dma_start(out=enc_ch[2], in_=encC_h[0, 3])
            for b in range(1, BPC):
                for lc in range(LCH):
                    nc.sync.dma_start(
                        out=enc_ch[4 * b + lc - 1], in_=encC_h[b, lc]
                    )

            # first ACT-queue op: anchors walrus's PSEUDO_LOAD_ACT_FUNC_SET
            # (the ~1.3us exp/tanh table load) at ~6.8us where it's free --
            # emitted before any tanh so the load never lands on the
            # critical path behind the first tanh's semaphore wait
            nc.scalar.copy(out=b1_sb, in_=b1_bf)

            dh_sb = wmisc_sb[:, 0:DH_LEN].rearrange("p (k b) -> p k b", k=KTILES)
            w2_sb = wmisc_sb[:, W2_OFF : W2_OFF + W2_LEN]

            decb = wp.tile([P, DTILES, BPC], f32, tag="decb")

            # PE clock-gate warmup: the HAM throttles the PE to half clock
            # after ~3.4us idle, and the runtime preamble + input DMA leave
            # the PE idle for ~9.5us at kernel start.  A short stream of
            # result-less matmuls (no readers; WAW-serialized on one spare
            # tile) keeps the PE active through the DMA wait so the HAM
            # un-throttles shortly after real matmuls begin.  Sized to drain
            # right as the first enc chunk lands (~9.6us); oversizing this
            # delays all real work (PE queue is FIFO).
            # memset on GpSimd: every engine's queue preamble ends ~5.1us,
            # and GpSimd is otherwise idle until the first softmax tail
            # (~21us), so the PE's first warmup matmul isn't queued behind
            # the DVE's busier early queue.
            #
            # The HAM SHORT window only un-throttles after ~3.4us of
            # CONTINUOUS PE busy (any idle pulse resets it -- a 75%-duty
            # WAW-paced warmup measurably never flips it), so the warmup
            # reuses ONE psum tile with no per-matmul reallocation: same-
            # engine WAW needs no semaphore and the matmuls stream at the
            # ~60ns issue rate with overlapping drains, i.e. truly gapless.
            # Sized (with the enc matmuls that follow seamlessly) to carry
            # the busy window past the flip at warmup_start + 3.4-6.8us.
            warmsrc = wp.tile([P, P], bf16, tag="warmsrc")
            nc.gpsimd.memset(warmsrc, 0.0)
            wps = decps.tile([P, BPC], f32, tag="d", name="warm")
            for w in range(72):
                nc.tensor.matmul(
                    out=wps, lhsT=warmsrc, rhs=warmsrc[:, 0:BPC]
                )

            def emit_dec(js):
                # dec_projT + b1 bias columns: [p, dtile, batch]
                for j in js:
                    dps = decps.tile([P, BPC], f32, tag="d", name=f"decps{j}")
                    for k in range(KTILES):
                        nc.tensor.matmul(
                            out=dps,
                            lhsT=w1d_j(j, k),
                            rhs=dh_sb[:, k, :],
                            start=(k == 0),
                            stop=(k == KTILES - 1),
                        )
                    nc.vector.tensor_scalar_add(
                        out=decb[:, j, :], in0=dps, scalar1=b1_sb[:, j : j + 1]
                    )

            # --- main loop over 2-chunk slots ---
            # Each slot computes TWO Lin-chunks: the four j-groups land in
            # [P, 2, LCHW] double-bank PSUM tiles (ring of 3) so ONE tanh
            # activation covers both chunks of a j (same per-partition
            # dec-bias), halving the ACT per-op overhead count.
            # Scores for batch b are emitted after batch b+1's first slot
            # as column-tiled quads: the 4 chunks' M=1 matmuls target
            # distinct 32-column groups (partitions 0/32/64/96), so the PE
            # array runs each quad's 4 streams concurrently.
            slots = [(b, h) for b in range(BPC) for h in range(LCH // 2)]
            scs_tiles = {}
            attn_tiles = {}
            sume_tiles = {}

            def emit_scores_batch(b, js=tuple(range(DTILES))):
                sc = scs_tiles[b]
                for j in js:
                    for lc in range(LCH):
                        at = attn_tiles[(b, lc // 2)][j]
                        nc.tensor.matmul(
                            out=sc[32 * lc : 32 * lc + 1, :],
                            lhsT=w2_sb[:, j : j + 1],
                            rhs=at[:, lc % 2, :],
                            start=(j == 0),
                            stop=(j == DTILES - 1),
                            tile_position=(0, 32 * lc),
                        )
                if js[-1] == DTILES - 1:
                    for h in range(LCH // 2):
                        attn_tiles.pop((b, h))

            def emit_exp(b):
                # one Exp for all 4 chunks (rows 0/32/64/96 + zeroed filler)
                erow = smp.tile([SCP, LCHW], f32, tag="erow", name=f"erow{b}")
                sume = smp.tile([SCP, 1], f32, tag="sume", name=f"sume{b}")
                nc.scalar.activation(
                    out=erow, in_=scs_tiles[b], func=AF.Exp, bias=0.0, scale=1.0,
                    accum_out=sume,
                )
                sume_tiles[b] = (erow, sume)

            def emit_tail2(b):
                # all-partition sum of per-chunk exp sums -> 1/sum -> scale
                erow, sume = sume_tiles.pop(b)
                scs_tiles.pop(b)
                sumall = smp.tile([SCP, 1], f32, tag="sumall", name=f"sumall{b}")
                nc.gpsimd.partition_all_reduce(
                    sumall, sume, SCP, bass_isa.ReduceOp.add
                )
                rinv97 = smp.tile([SCP, 1], f32, tag="rinv97", name=f"rinv97{b}")
                nc.vector.reciprocal(out=rinv97, in_=sumall)
                orow = smp.tile([SCP, LCHW], f32, tag="orow", name=f"orow{b}")
                nc.vector.tensor_scalar_mul(out=orow, in0=erow, scalar1=rinv97)
                nc.sync.dma_start(
                    out=out_h[b : b + 1, :].rearrange("o (c w) -> o c w", c=LCH),
                    in_=orow[0 : 3 * 32 + 1 : 32, :],
                )

            for i, (b, h) in enumerate(slots):
                ca, cb = 2 * h, 2 * h + 1
                mpss = []
                for j in range(DTILES):
                    mps = mainps.tile(
                        [P, 2, LCHW], f32, tag="m", name=f"mps_b{b}h{h}j{j}"
                    )
                    for c in (0, 1):
                        for t in range(ETILES // 2):
                            nc.tensor.matmul(
                                out=mps[:, c, :],
                                lhsT=w1e_sb[:, 2 * t : 2 * t + 2, ts(j, P)],
                                rhs=enc_tiles[b][ca + c][:, 2 * t : 2 * t + 2, :],
                                start=(t == 0),
                                stop=(t == ETILES // 2 - 1),
                                perf_mode=DR,
                            )
                    mpss.append(mps)
                    if i == 0:
                        # weave dec j-groups between slot-0 enc chunk groups:
                        # decb[j] is ready right before tanh j needs it, the
                        # decps WAW reuse stall hides under enc matmuls, and
                        # the PE stays dense enough that the HAM never sees
                        # an idle MID window here
                        emit_dec((j,))
                if h == 0 and b >= 1:
                    emit_scores_batch(b - 1)
                if h == 1:
                    if b >= 1:
                        emit_tail2(b - 1)
                    # score bank for batch b; its only gen-(b-1) reader is
                    # exp(b-1), one slot back — must precede scores(b) quads
                    sc = scpsp.tile([SCP, LCHW], f32, tag="sc", name=f"sc{b}")
                    scs_tiles[b] = sc
                    nc.vector.memset(sc, -100.0)

                attns = []
                for j in range(DTILES):
                    at = attnp.tile(
                        [P, 2, LCHW], bf16, tag="attn", name=f"attn_b{b}h{h}j{j}"
                    )
                    nc.scalar.activation(
                        out=at,
                        in_=mpss[j],
                        func=AF.Tanh,
                        bias=decb[:, j, b : b + 1],
                        scale=INV_SCALE,
                    )
                    attns.append(at)
                attn_tiles[(b, h)] = attns
                if h == 0 and b >= 1:
                    # emitted after this slot's tanhs so the in-order ACT
                    # queue never parks on the Exp while tanh work is ready
                    emit_exp(b - 1)
                if i == len(slots) - 1:
                    # last batch: j0/j1 quads run as soon as this slot's
                    # early tanhs land, shortening the tail
                    emit_scores_batch(b, (0, 1))

            b_last = BPC - 1
            emit_scores_batch(b_last, (2, 3))
            emit_exp(b_last)
            emit_tail2(b_last)
    nc.compile()
    return nc


def _prep_in_maps(d_hidden, encoder_outputs, W1, b1, w2):
    import ml_dtypes

    bf = ml_dtypes.bfloat16
    f8 = ml_dtypes.float8_e4m3
    d_hidden = np.ascontiguousarray(np.asarray(d_hidden), dtype=np.float32)
    encoder_outputs = np.asarray(encoder_outputs)
    W1 = np.ascontiguousarray(np.asarray(W1), dtype=np.float32)
    b1 = np.ascontiguousarray(np.asarray(b1), dtype=np.float32)
    w2 = np.ascontiguousarray(np.asarray(w2), dtype=np.float32)

    W1d, W1e = W1[:ND], W1[ND:]
    w1e8 = np.ascontiguousarray(
        (W1e * W1E_SCALE)
        .reshape(ETILES, P, D)
        .transpose(1, 0, 2)
        .reshape(P, W1E_LEN)
        .astype(f8)
    )
    w1dk = W1d.reshape(KTILES, P, D).transpose(1, 0, 2).astype(bf)  # [P, k, D]
    w1d0b = np.concatenate(
        [
            w1dk[:, :, :P].reshape(P, KTILES * P),
            b1.reshape(DTILES, P).T.astype(bf),
        ],
        axis=1,
    )
    w1dj1 = np.ascontiguousarray(w1dk[:, :, P : 2 * P])
    w1d1 = np.ascontiguousarray(w1dk[:, :, DHALF:])

    in_maps = []
    for c in range(NCORES):
        bs = slice(c * BPC, (c + 1) * BPC)
        encT = (
            np.asarray(encoder_outputs[bs], dtype=np.float32).transpose(0, 2, 1)
            * ENC_SCALE
        )  # [BPC, E, LIN] scaled
        encC = np.ascontiguousarray(
            encT.reshape(BPC, ETILES, P, LCH, LCHW)
            .transpose(0, 3, 2, 1, 4)
            .astype(f8)
        )
        dhT = np.ascontiguousarray(d_hidden[bs].reshape(BPC, ND).T)  # [ND, BPC]
        wmisc = np.zeros((P, WMISC), dtype=bf)
        wmisc[:, 0:DH_LEN] = (
            dhT.reshape(KTILES, P, BPC).transpose(1, 0, 2).reshape(P, DH_LEN)
            .astype(bf)
        )
        wmisc[:, W2_OFF : W2_OFF + W2_LEN] = w2.reshape(DTILES, P).T.astype(bf)
        head8 = np.concatenate(
            [w1e8, encC[0, 0].reshape(P, ETILES * LCHW)], axis=1
        )
        in_maps.append(
            {
                "encC": encC,
                "head8": head8,
                "wmisc": wmisc,
                "w1d0b": w1d0b,
                "w1dj1": w1dj1,
                "w1d1": w1d1,
            }
        )
    return in_maps


def kernel(d_hidden, encoder_outputs, W1, b1, w2):
    global LAST_RESULT
    from concourse import bass_utils

    if "nc" not in _CACHE:
        _CACHE["nc"] = _build()
    nc = _CACHE["nc"]

    in_maps = _prep_in_maps(d_hidden, encoder_outputs, W1, b1, w2)
    res = bass_utils.run_bass_kernel_spmd(
        nc,
        in_maps,
        core_ids=list(range(NCORES)),
        trace=TRACE,
        **TRACE_KWARGS,
    )
    LAST_RESULT = res
    return np.concatenate([r["out"] for r in res.results], axis=0)



# revision 20
# speedup vs baseline: 1.0517x; 1.0179x over previous
"""Bass/Trainium2 kernel for nn_Attention_42305427865835.

Computes, for d_hidden [B,N,D], encoder_outputs [B,Lin,E], W1 [E+N*D, D],
b1 [D], w2 [D]:
    dec_proj = d_flat @ W1[:N*D] + b1                    # [B, D]
    enc_proj = enc @ W1[N*D:]                            # [B, Lin, E->D]
    scores   = tanh(enc_proj + dec_proj[:,None,:]) @ w2  # [B, Lin]
    out      = softmax(scores, axis=-1)
sharded data-parallel over batch, 4 batches per core on 8 cores.

Device-side layout is transposed ("T layout": D/E on partitions, Lin on the
free axis) so the contraction over E maps onto the PE array and the
dec_proj/b1 bias-add rides the ScalarE activation's per-partition bias.

The enc matmul (the dominant FLOPs) runs in fp8e4 with
MatmulPerfMode.DoubleRow: host pre-scales enc by 32 and W1_e by 8192
(keeping both inside fp8e4's +-240 range), packs the contraction as
[P, etile, free] so an e-tile PAIR is one K=256 DoubleRow matmul, and the
tanh activation's scale=2^-18 undoes the scaling exactly.  The score matmul
stays bf16 (fp8 there would blow the error budget).  Simulated end-to-end
absmax-relative error 1.83e-2 (gate 2e-2); the same simulator matches the
bf16 baseline's hardware error to 3 digits.

Softmax: scores for the 4 Lin-chunks of a batch land on PSUM partitions
{0,32,64,96} of one bank (tile_position picks the column group), so ONE Exp
activation covers the whole batch and its accum_out gives per-chunk sums.
The bank is memset to -100 first so unused partitions exp to 0, making the
ones-vector partition-sum matmul exact; gpsimd.partition_broadcast spreads
1/sum back across partitions for the final scale.  The partition-sum matmul
reuses element [0,0] of the score bank (no spare PSUM bank exists), and the
tail is pipelined across two chunk slots so the PE never waits on the Exp.

Score matmuls are emitted one chunk behind the enc matmuls so the PE queue
never head-blocks on the tanh that produces their input.  W1_d comes in two
half-tensors (d columns 0:256 / 256:512) so the dec matmuls can start after
only half the weight bytes have landed.

All enc chunks transfer as individual [P, 2048-byte] DMAs in consumption
order (whole-batch DMAs only signal on the last byte, which starved the PE
~3.4us at each batch handoff and re-throttled the HAM clock gate); the dec
matmuls weave between slot-0 enc chunk groups so decb[j] lands just before
tanh j first needs it and the PE has no idle window the HAM could see.

Softmax skips the max-subtraction: |scores| <= ||w2||_1 ~ 11, well inside
exp's fp32 range.
"""

import numpy as np

B, LIN, E, D, N = 32, 2048, 512, 512, 2
NCORES = 8
BPC = B // NCORES      # batches per core
P = 128                # SBUF partitions
ETILES = E // P        # 4
DTILES = D // P        # 4
ND = N * D             # 1024
KTILES = ND // P       # 8
LCHW = 512             # Lin chunk width (one PSUM bank of fp32)
LCH = LIN // LCHW      # 4

ENC_SCALE = 32.0       # enc pre-scale into fp8e4
W1E_SCALE = 8192.0     # W1_e pre-scale into fp8e4
INV_SCALE = 1.0 / (ENC_SCALE * W1E_SCALE)   # 2^-18, exact

# wmisc (bf16): dec-hidden columns + w2 columns
DH_LEN = KTILES * BPC          # 32: [k, b] -> d_flat[b, k*P+p]
W2_OFF = DH_LEN
W2_LEN = DTILES                # 4:  [a]    -> w2[a*P+p]
WMISC = DH_LEN + W2_LEN        # 36
DHALF = D // 2                 # 256
W1E_LEN = ETILES * D           # 2048: [e, d] -> W1_e[e*P+p, d] (fp8)

SCP = 3 * 32 + 1               # 97: score rows live at partitions {0,32,64,96}

TRACE = False
TRACE_KWARGS = {}
LAST_RESULT = None

_CACHE = {}


def _build():
    import concourse.bacc as bacc
    import concourse.mybir as mybir
    import concourse.tile as tile
    from concourse.bass import ts

    from concourse import bass_isa

    f32 = mybir.dt.float32
    bf16 = mybir.dt.bfloat16
    fp8 = mybir.dt.float8e4
    AF = mybir.ActivationFunctionType
    DR = mybir.MatmulPerfMode.DoubleRow

    nc = bacc.Bacc("TRN2", target_bir_lowering=False)

    encC_h = nc.dram_tensor(
        "encC", [BPC, LCH, P, ETILES, LCHW], fp8, kind="ExternalInput"
    )
    head8_h = nc.dram_tensor("head8", [P, 2 * W1E_LEN], fp8, kind="ExternalInput")
    wmisc_h = nc.dram_tensor("wmisc", [P, WMISC], bf16, kind="ExternalInput")
    # w1d0b: W1_d columns for dec j0 (d 0:128) ++ b1 columns, all bf16
    w1d0b_h = nc.dram_tensor(
        "w1d0b", [P, KTILES * P + DTILES], bf16, kind="ExternalInput"
    )
    # w1dj1: W1_d columns for dec j1 (d 128:256)
    w1dj1_h = nc.dram_tensor("w1dj1", [P, KTILES, P], bf16, kind="ExternalInput")
    w1d1_h = nc.dram_tensor("w1d1", [P, KTILES, DHALF], bf16, kind="ExternalInput")
    out_h = nc.dram_tensor("out", [BPC, LIN], f32, kind="ExternalOutput")

    with tile.TileContext(nc) as tc:
        with (
            tc.tile_pool(name="persist", bufs=1) as wp,
            tc.tile_pool(name="encp", bufs=LCH * BPC - 1) as encp,
            tc.tile_pool(name="attnp", bufs=20) as attnp,
            tc.tile_pool(name="smp", bufs=2) as smp,
            tc.tile_pool(name="mainps", bufs=3, space="PSUM") as mainps,
            tc.tile_pool(name="scpsp", bufs=2, space="PSUM") as scpsp,
        ):
            # scpsp holds every 1-bank psum tile, time-shared through one
            # tag (warmup target -> dec partials -> score banks): 6 mainps
            # banks + these 2 fill PSUM exactly, and the bufs=2 rotation
            # double-buffers the score banks (sc(b) no longer waits on
            # exp(b-1) the way a single dedicated bank did).
            # --- critical path: w1e + first enc chunk fused in ONE DMA ---
            head_sb = wp.tile([P, 2 * ETILES, LCHW], fp8, tag="head8")
            nc.sync.dma_start(
                out=head_sb, in_=head8_h.rearrange("p (e d) -> p e d", e=2 * ETILES)
            )
            w1e_sb = head_sb[:, 0:ETILES, :]

            # all enc chunks land as individual [P, ETILES, LCHW] DMAs so
            # compute unblocks at chunk granularity (a whole-batch DMA only
            # signals when its last byte lands, which starved the PE for
            # ~3.4us at the batch 0 -> 1 handoff and re-throttled the HAM)
            enc_ch = [
                encp.tile(
                    [P, ETILES, LCHW], fp8, tag="enc", name=f"enc{b}c{lc}"
                )
                for b in range(BPC)
                for lc in range(LCH)
                if (b, lc) != (0, 0)
            ]
            enc_tiles = [[head_sb[:, ETILES : 2 * ETILES, :]] + enc_ch[0:3]] + [
                enc_ch[4 * b - 1 : 4 * b + 3] for b in range(1, BPC)
            ]

            w1d0b_sb = wp.tile([P, KTILES * P + DTILES], bf16, tag="w1d0b")
            w1dj1_sb = wp.tile([P, KTILES, P], bf16, tag="w1dj1", name="w1dj1")
            w1d1_sb = wp.tile([P, KTILES, DHALF], bf16, tag="w1d1", name="w1d1")
            w1dj0 = w1d0b_sb[:, 0 : KTILES * P].rearrange("p (k d) -> p k d", k=KTILES)

            def w1d_j(j, k):  # lhsT [P, 128] for dec j-tile, k-tile
                if j == 0:
                    return w1dj0[:, k]
                if j == 1:
                    return w1dj1_sb[:, k]
                return w1d1_sb[:, k, ts(j - 2, P)]

            b1_bf = w1d0b_sb[:, KTILES * P :]
            b1_sb = wp.tile([P, DTILES], f32, tag="b1f")
            wmisc_sb = wp.tile([P, WMISC], bf16, tag="wmisc")

            # trigger order == consumption order, with the dec weights split
            # per j-tile so each piece is small enough not to starve the
            # head/chunk stream it shares HBM with: each dec j-group's
            # weights land just before the woven dec matmuls need them.
            # b0c1 goes before the dec weights: slot (0,h0) covers chunks
            # 0+1, so b0c1 gates the very first enc matmul group.
            nc.sync.dma_start(out=wmisc_sb, in_=wmisc_h[:, :])
            nc.sync.dma_start(out=enc_ch[0], in_=encC_h[0, 1])
            nc.sync.dma_start(out=w1d0b_sb, in_=w1d0b_h[:, :])
            nc.sync.dma_start(out=enc_ch[1], in_=encC_h[0, 2])
            nc.sync.dma_start(out=w1dj1_sb, in_=w1dj1_h[:, :, :])
            nc.sync.dma_start(out=enc_ch[2], in_=encC_h[0, 3])
            nc.sync.dma_start(out=w1d1_sb, in_=w1d1_h[:, :, :])
            for b in range(1, BPC):
                for lc in range(LCH):
                    nc.sync.dma_start(
                        out=enc_ch[4 * b + lc - 1], in_=encC_h[b, lc]
                    )

            # first ACT-queue op: anchors walrus's PSEUDO_LOAD_ACT_FUNC_SET
            # (the ~1.3us exp/tanh table load) at ~6.8us where it's free --
            # emitted before any tanh so the load never lands on the
            # critical path behind the first tanh's semaphore wait
            nc.scalar.copy(out=b1_sb, in_=b1_bf)

            dh_sb = wmisc_sb[:, 0:DH_LEN].rearrange("p (k b) -> p k b", k=KTILES)
            w2_sb = wmisc_sb[:, W2_OFF : W2_OFF + W2_LEN]

            decb = wp.tile([P, DTILES, BPC], f32, tag="decb")

            # PE clock-gate warmup: the HAM throttles the PE to half clock
            # and only un-throttles after ~3.4us of CONTINUOUS PE-array
            # busy -- LDWEIGHTS time does not count, and any idle pulse
            # resets the window (a 75%-duty N=4 warmup measurably never
            # flips it).  N=512 matmuls have dur > issue spacing even with
            # the per-matmul weight reload exposed, so a short chain of
            # them is truly gapless; the enc matmuls that follow (also
            # overlapping) carry the busy window through the flip at
            # warmup_start + 3.4-6.8us.  Sized to drain right as b0c1
            # lands (~11.2us); oversizing delays real work (FIFO queue).
            warmsrc = wp.tile([P, LCHW], bf16, tag="warmsrc")
            nc.vector.memset(warmsrc, 0.0)
            wps = scpsp.tile([P, LCHW], f32, tag="sc", name="warm")
            for w in range(7):
                nc.tensor.matmul(
                    out=wps, lhsT=warmsrc[:, 0:P], rhs=warmsrc
                )

            def emit_dec(js):
                # dec_projT + b1 bias columns: [p, dtile, batch]
                for j in js:
                    dpt = scpsp.tile([P, LCHW], f32, tag="sc", name=f"decps{j}")
                    dps = dpt[:, 0:BPC]
                    for k in range(KTILES):
                        nc.tensor.matmul(
                            out=dps,
                            lhsT=w1d_j(j, k),
                            rhs=dh_sb[:, k, :],
                            start=(k == 0),
                            stop=(k == KTILES - 1),
                        )
                    nc.vector.tensor_scalar_add(
                        out=decb[:, j, :], in0=dps, scalar1=b1_sb[:, j : j + 1]
                    )

            # --- main loop over 2-chunk slots ---
            # Each slot computes TWO Lin-chunks: the four j-groups land in
            # [P, 2, LCHW] double-bank PSUM tiles (ring of 3) so ONE tanh
            # activation covers both chunks of a j (same per-partition
            # dec-bias), halving the ACT per-op overhead count.
            # Scores for batch b are emitted after batch b+1's first slot
            # as column-tiled quads: the 4 chunks' M=1 matmuls target
            # distinct 32-column groups (partitions 0/32/64/96), so the PE
            # array runs each quad's 4 streams concurrently.
            slots = [(b, h) for b in range(BPC) for h in range(LCH // 2)]
            scs_tiles = {}
            attn_tiles = {}
            sume_tiles = {}

            def emit_scores_batch(b, js=tuple(range(DTILES))):
                sc = scs_tiles[b]
                for j in js:
                    for lc in range(LCH):
                        at = attn_tiles[(b, lc // 2)][j]
                        nc.tensor.matmul(
                            out=sc[32 * lc : 32 * lc + 1, :],
                            lhsT=w2_sb[:, j : j + 1],
                            rhs=at[:, lc % 2, :],
                            start=(j == 0),
                            stop=(j == DTILES - 1),
                            tile_position=(0, 32 * lc),
                        )
                if js[-1] == DTILES - 1:
                    for h in range(LCH // 2):
                        attn_tiles.pop((b, h))

            def emit_exp(b):
                # one Exp for all 4 chunks (rows 0/32/64/96 + zeroed filler).
                # Mid-kernel batches sum erow on the DVE (idle) instead of
                # via accum_out: the ACTIVATION_READ_ACCUMULATOR it forces
                # is an ACT-queue instruction that costs ~0.5us/batch of
                # tanh time.  The last batch keeps accum_out -- there the
                # read (284ns) beats a DVE reduce (~660ns) on the exposed
                # softmax tail.
                erow = smp.tile([SCP, LCHW], f32, tag="erow", name=f"erow{b}")
                sume = smp.tile([SCP, 1], f32, tag="sume", name=f"sume{b}")
                if b == BPC - 1:
                    nc.scalar.activation(
                        out=erow, in_=scs_tiles[b], func=AF.Exp, bias=0.0,
                        scale=1.0, accum_out=sume,
                    )
                else:
                    nc.scalar.activation(
                        out=erow, in_=scs_tiles[b], func=AF.Exp, bias=0.0,
                        scale=1.0,
                    )
                    nc.vector.reduce_sum(
                        out=sume, in_=erow, axis=mybir.AxisListType.X
                    )
                sume_tiles[b] = (erow, sume)

            def emit_tail2(b):
                # all-partition sum of per-chunk exp sums -> 1/sum -> scale
                erow, sume = sume_tiles.pop(b)
                scs_tiles.pop(b)
                sumall = smp.tile([SCP, 1], f32, tag="sumall", name=f"sumall{b}")
                nc.gpsimd.partition_all_reduce(
                    sumall, sume, SCP, bass_isa.ReduceOp.add
                )
                rinv97 = smp.tile([SCP, 1], f32, tag="rinv97", name=f"rinv97{b}")
                nc.vector.reciprocal(out=rinv97, in_=sumall)
                orow = smp.tile([SCP, LCHW], f32, tag="orow", name=f"orow{b}")
                nc.vector.tensor_scalar_mul(out=orow, in0=erow, scalar1=rinv97)
                nc.sync.dma_start(
                    out=out_h[b : b + 1, :].rearrange("o (c w) -> o c w", c=LCH),
                    in_=orow[0 : 3 * 32 + 1 : 32, :],
                )

            for i, (b, h) in enumerate(slots):
                ca, cb = 2 * h, 2 * h + 1
                mpss = []
                for j in range(DTILES):
                    mps = mainps.tile(
                        [P, 2, LCHW], f32, tag="m", name=f"mps_b{b}h{h}j{j}"
                    )
                    for c in (0, 1):
                        for t in range(ETILES // 2):
                            nc.tensor.matmul(
                                out=mps[:, c, :],
                                lhsT=w1e_sb[:, 2 * t : 2 * t + 2, ts(j, P)],
                                rhs=enc_tiles[b][ca + c][:, 2 * t : 2 * t + 2, :],
                                start=(t == 0),
                                stop=(t == ETILES // 2 - 1),
                                perf_mode=DR,
                            )
                    mpss.append(mps)
                    if i == 0:
                        # weave dec j-groups between slot-0 enc chunk groups:
                        # decb[j] is ready right before tanh j needs it, the
                        # decps WAW reuse stall hides under enc matmuls, and
                        # the PE stays dense enough that the HAM never sees
                        # an idle MID window here
                        emit_dec((j,))
                if h == 0 and b >= 1:
                    emit_scores_batch(b - 1)
                if h == 1:
                    if b >= 1:
                        emit_tail2(b - 1)
                    # score bank for batch b; its only gen-(b-1) reader is
                    # exp(b-1), one slot back — must precede scores(b) quads
                    sc = scpsp.tile([SCP, LCHW], f32, tag="sc", name=f"sc{b}")
                    scs_tiles[b] = sc
                    nc.vector.memset(sc, -100.0)

                attns = []
                for j in range(DTILES):
                    at = attnp.tile(
                        [P, 2, LCHW], bf16, tag="attn", name=f"attn_b{b}h{h}j{j}"
                    )
                    nc.scalar.activation(
                        out=at,
                        in_=mpss[j],
                        func=AF.Tanh,
                        bias=decb[:, j, b : b + 1],
                        scale=INV_SCALE,
                    )
                    attns.append(at)
                attn_tiles[(b, h)] = attns
                if h == 0 and b >= 1:
                    # emitted after this slot's tanhs so the in-order ACT
                    # queue never parks on the Exp while tanh work is ready
                    emit_exp(b - 1)
                if i == len(slots) - 1:
                    # last batch: j0/j1 quads run as soon as this slot's
                    # early tanhs land, shortening the tail
                    emit_scores_batch(b, (0, 1))

            b_last = BPC - 1
            emit_scores_batch(b_last, (2, 3))
            emit_exp(b_last)
            emit_tail2(b_last)
    nc.compile()
    return nc


def _prep_in_maps(d_hidden, encoder_outputs, W1, b1, w2):
    import ml_dtypes

    bf = ml_dtypes.bfloat16
    f8 = ml_dtypes.float8_e4m3
    d_hidden = np.ascontiguousarray(np.asarray(d_hidden), dtype=np.float32)
    encoder_outputs = np.asarray(encoder_outputs)
    W1 = np.ascontiguousarray(np.asarray(W1), dtype=np.float32)
    b1 = np.ascontiguousarray(np.asarray(b1), dtype=np.float32)
    w2 = np.ascontiguousarray(np.asarray(w2), dtype=np.float32)

    W1d, W1e = W1[:ND], W1[ND:]
    w1e8 = np.ascontiguousarray(
        (W1e * W1E_SCALE)
        .reshape(ETILES, P, D)
        .transpose(1, 0, 2)
        .reshape(P, W1E_LEN)
        .astype(f8)
    )
    w1dk = W1d.reshape(KTILES, P, D).transpose(1, 0, 2).astype(bf)  # [P, k, D]
    w1d0b = np.concatenate(
        [
            w1dk[:, :, :P].reshape(P, KTILES * P),
            b1.reshape(DTILES, P).T.astype(bf),
        ],
        axis=1,
    )
    w1dj1 = np.ascontiguousarray(w1dk[:, :, P : 2 * P])
    w1d1 = np.ascontiguousarray(w1dk[:, :, DHALF:])

    in_maps = []
    for c in range(NCORES):
        bs = slice(c * BPC, (c + 1) * BPC)
        encT = (
            np.asarray(encoder_outputs[bs], dtype=np.float32).transpose(0, 2, 1)
            * ENC_SCALE
        )  # [BPC, E, LIN] scaled
        encC = np.ascontiguousarray(
            encT.reshape(BPC, ETILES, P, LCH, LCHW)
            .transpose(0, 3, 2, 1, 4)
            .astype(f8)
        )
        dhT = np.ascontiguousarray(d_hidden[bs].reshape(BPC, ND).T)  # [ND, BPC]
        wmisc = np.zeros((P, WMISC), dtype=bf)
        wmisc[:, 0:DH_LEN] = (
            dhT.reshape(KTILES, P, BPC).transpose(1, 0, 2).reshape(P, DH_LEN)
            .astype(bf)
        )
        wmisc[:, W2_OFF : W2_OFF + W2_LEN] = w2.reshape(DTILES, P).T.astype(bf)
        head8 = np.concatenate(
            [w1e8, encC[0, 0].reshape(P, ETILES * LCHW)], axis=1
        )
        in_maps.append(
            {
                "encC": encC,
                "head8": head8,
                "wmisc": wmisc,
                "w1d0b": w1d0b,
                "w1dj1": w1dj1,
                "w1d1": w1d1,
            }
        )
    return in_maps


def kernel(d_hidden, encoder_outputs, W1, b1, w2):
    global LAST_RESULT
    from concourse import bass_utils

    if "nc" not in _CACHE:
        _CACHE["nc"] = _build()
    nc = _CACHE["nc"]

    in_maps = _prep_in_maps(d_hidden, encoder_outputs, W1, b1, w2)
    res = bass_utils.run_bass_kernel_spmd(
        nc,
        in_maps,
        core_ids=list(range(NCORES)),
        trace=TRACE,
        **TRACE_KWARGS,
    )
    LAST_RESULT = res
    return np.concatenate([r["out"] for r in res.results], axis=0)

